# revision 22
# baseline (speedup 1.0000x reference)
"""Trainium2 Bass kernel for nn_CCALoss (CLIP + masked concept BCE + Jaccard-KL loss).

Contract: kernel(**inputs) takes the FULL unsharded inputs and returns the FULL
(scalar) output. Internally shards the batch dim across 8 NeuronCores; each core
computes per-row partial sums; the host does the O(B) finalization in fp64.

Per-core device work (R = 512 local rows, B = 4096, C = 512):
  - Zimg/Ztxt:  row-wise sum(exp(x)) of logits_per_image/text local rows
                (ScalarE exp with fused accum_out; lse computed on host).
  - BCE:        masked softplus sums over concepts for local rows
                (exp + log1p on ScalarE, fused STT dot-accumulate on VectorE).
  - Jaccard:    inter/union via two GEMMs over u=(mc!=0), v=(mc==1) in bf16 on
                TensorE. union = rs_i + rs_j - inter accumulated directly in
                PSUM via K=1 fp16 fold matmuls. q = 1/union (custom DVE recip),
                s' = (rs_i+rs_j)*q - 1, e = exp(s'/T) (ScalarE, accum -> Zs),
                ES = sum e*s' and EC = sum e*csim via fused STT accumulates.
"""

import numpy as np

import concourse.bacc as bacc
import concourse.bass as bass
import concourse.tile as tile
from concourse import mybir
from concourse.bass_utils import run_bass_kernel_spmd

B = 4096
C = 512
NCORES = 8
R = B // NCORES  # 512 rows per core
TEMP = 0.07
CONCEPT_WEIGHT = 0.5
CONCEPT_SIM_WEIGHT = 0.3

FP32 = mybir.dt.float32
FP8 = mybir.dt.float8e4
FP16 = mybir.dt.float16
BF16 = mybir.dt.bfloat16
I32 = mybir.dt.int32
AX = mybir.AxisListType
ALU = mybir.AluOpType
ACTF = mybir.ActivationFunctionType

# output rows in the [NROWS, 4, 128] per-core output tensor
O_ZIMG, O_ZTXT, O_ZC, O_ZS, O_ES, O_EC, O_B1, O_B2, O_MS = range(9)
NROWS = 9


def build_nc():
    nc = bacc.Bacc("TRN2", target_bir_lowering=False, debug=False)

    img = nc.dram_tensor("img", [R, B], FP32, kind="ExternalInput")
    txt = nc.dram_tensor("txt", [R, B], FP32, kind="ExternalInput")
    csim = nc.dram_tensor("csim", [R, B], FP32, kind="ExternalInput")
    mcf = nc.dram_tensor("mcf", [B, C], I32, kind="ExternalInput")
    mcl = nc.dram_tensor("mcl", [R, C], I32, kind="ExternalInput")
    clog = nc.dram_tensor("clog", [R, C], FP32, kind="ExternalInput")
    out = nc.dram_tensor("out", [NROWS, 4, 128], FP32, kind="ExternalOutput")

    # internal DRAM scratch for partition-major <-> free-major relayouts
    rs_d = nc.dram_tensor("rs_scratch", [B], FP16)
    rsl_d = nc.dram_tensor("rsl_scratch", [R], FP32)

    with tile.TileContext(nc) as tc:
        _build(nc, tc, img, txt, csim, mcf, mcl, clog, out, rs_d, rsl_d)
    nc.compile()
    return nc


def _build(nc, tc, img, txt, csim, mcf, mcl, clog, out, rs_d, rsl_d):
    from contextlib import ExitStack

    ctx = ExitStack()
    with ctx:
        singles = ctx.enter_context(tc.tile_pool(name="singles", bufs=1))
        mc_pool = ctx.enter_context(tc.tile_pool(name="mc", bufs=6))
        big = ctx.enter_context(tc.tile_pool(name="big", bufs=3))
        cs_pool = ctx.enter_context(tc.tile_pool(name="cs", bufs=5))
        s3 = ctx.enter_context(tc.tile_pool(name="s3", bufs=3))
        scrp = ctx.enter_context(tc.tile_pool(name="scr", bufs=1))
        bce_pool = ctx.enter_context(tc.tile_pool(name="bce", bufs=1))
        stats = ctx.enter_context(tc.tile_pool(name="stats", bufs=1))
        ps_main = ctx.enter_context(tc.tile_pool(name="psA", bufs=2, space="PSUM"))
        ps_rs = ctx.enter_context(tc.tile_pool(name="psB", bufs=2, space="PSUM"))

        # ---------------- constants ----------------
        ones16 = singles.tile([128, 512], FP16)
        nc.vector.memset(ones16, 1.0)
        mones_bf = singles.tile([128, 1], BF16)
        nc.vector.memset(mones_bf, -1.0)
        halves_bf = singles.tile([128, 1], BF16)
        nc.vector.memset(halves_bf, 0.5)
        one_col = singles.tile([128, 1], FP32)
        nc.vector.memset(one_col, 1.0)
        minvt_col = singles.tile([128, 1], FP32)
        nc.vector.memset(minvt_col, -float(1.0 / TEMP))

        # persistent big tensors
        # u_T4[p, rc, cc, j] = u[rc*128+j, cc*128+p]
        u_T4 = singles.tile([128, 32, 4, 128], BF16)
        v_T4 = singles.tile([128, 32, 4, 128], BF16)
        nu4 = singles.tile([128, 4, 4, 128], BF16)  # -0.5 * u_local^T
        nv4 = singles.tile([128, 4, 4, 128], BF16)
        rs_bcast = singles.tile([128, B], FP16)  # row sums of w, bcast on parts
        rsrow_sb = rs_bcast[0:1, :]  # free-major rs staging = partition 0
        rsloc_sb = singles.tile([128, 4], FP32)  # rs of local rows, part-major
        rsif = singles.tile([1, R], FP32)  # rs of local rows, free-major
        rsif16 = singles.tile([1, R], FP16)

        # per-row stats tiles (partition-major, col = row-tile index)
        parts = {
            k: stats.tile([128, 4], FP32, tag=f"p{k}", name=f"parts{k}")
            for k in range(NROWS)
        }

        # ---------------- phase 1: u_T / v_T from full mc ----------------
        # u = (mc != 0), v = (mc == 1) as bf16 {0,1}; per-row-chunk sbuf
        # xbar transposes into contiguous [4,128] dests. Bacc's
        # generate_event_semaphores pass collapses multi-waits so the
        # single-wait-slot XPOSE instructions lower fine.
        uvpre = ctx.enter_context(tc.tile_pool(name="uvpre", bufs=4))
        for rc in range(32):
            mct = mc_pool.tile([128, C], I32, tag="mcf", name=f"mcf{rc}")
            nc.sync.dma_start(out=mct, in_=mcf[rc * 128:(rc + 1) * 128, :])
            up = uvpre.tile([128, C], BF16, tag="up", name=f"up{rc}")
            vp = uvpre.tile([128, C], BF16, tag="vp", name=f"vp{rc}")
            nc.vector.tensor_scalar(up, mct, 0, None, ALU.not_equal)
            nc.vector.tensor_scalar(vp, mct, 0, None, ALU.max)
            nc.sync.dma_start_transpose(out=u_T4[:, rc], in_=up)
            nc.scalar.dma_start_transpose(out=v_T4[:, rc], in_=vp)
        for ic in range(4):
            mct = mc_pool.tile([128, C], I32, tag="mcf", name=f"mcln{ic}")
            nc.sync.dma_start(out=mct, in_=mcl[ic * 128:(ic + 1) * 128, :])
            nup = uvpre.tile([128, C], BF16, tag="up", name=f"nup{ic}")
            nvp = uvpre.tile([128, C], BF16, tag="vp", name=f"nvp{ic}")
            nc.vector.tensor_scalar(nup, mct, 0, -0.5, ALU.not_equal, ALU.mult)
            nc.vector.tensor_scalar(nvp, mct, 1, -0.5, ALU.is_equal, ALU.mult)
            nc.sync.dma_start_transpose(out=nu4[:, ic], in_=nup)
            nc.scalar.dma_start_transpose(out=nv4[:, ic], in_=nvp)

        # ---------------- phase 3: row-sum vectors rs ----------------
        # rs_loc[i] (partition-major) = sum_c 0.5*(u+v) for local rows
        for ic in range(4):
            ps = ps_rs.tile([128, 1], FP32, tag="rsloc")
            k = 0
            for loc in (nu4, nv4):
                for cc in range(4):
                    nc.tensor.matmul(
                        ps, loc[:, ic, cc, :], mones_bf,
                        start=(k == 0), stop=(k == 7))
                    k += 1
            nc.scalar.copy(rsloc_sb[:, ic:ic + 1], ps)
        # relayout partition-major -> free-major through DRAM
        nc.gpsimd.dma_start(
            out=rsl_d.ap().rearrange("(t p) -> p t", p=128), in_=rsloc_sb)
        nc.gpsimd.dma_start(
            out=rsif, in_=rsl_d.ap().rearrange("(o x) -> o x", o=1))
        nc.vector.tensor_copy(rsif16, rsif)

        # rs_row[j] for all 4096 j (free-major): ones-reduce over u_T/v_T
        for js in range(8):
            ps = ps_rs.tile([1, 512], FP32, tag="rsrow")
            k = 0
            for tens in (u_T4, v_T4):
                for cc in range(4):
                    nc.tensor.matmul(
                        ps, halves_bf, tens[:, 4 * js:4 * js + 4, cc, :],
                        start=(k == 0), stop=(k == 7))
                    k += 1
            nc.scalar.copy(rsrow_sb[:, js * 512:(js + 1) * 512], ps)
        nc.gpsimd.dma_start(
            out=rs_d.ap().rearrange("(o x) -> o x", o=1), in_=rsrow_sb)
        rs_bc_src = bass.AP(
            tensor=rs_d.ap().tensor, offset=0, ap=[[0, 127], [1, B]])
        nc.gpsimd.dma_start(out=rs_bcast[1:128, :], in_=rs_bc_src)

        # ---------------- phase 2: img/txt exp-sum streams ----------------
        for src, orow in ((img, O_ZIMG), (txt, O_ZTXT)):
            for t in range(4):
                acc = stats.tile([128, 2], FP32, tag="zacc")
                for h in range(2):
                    tl = big.tile([128, 2048], FP32, tag="imgtxt")
                    nc.sync.dma_start(
                        out=tl,
                        in_=src[t * 128:(t + 1) * 128, h * 2048:(h + 1) * 2048],
                    )
                    # in-place exp; we only need the row-sum accumulator
                    nc.scalar.activation(tl, tl, ACTF.Exp, accum_out=acc[:, h:h + 1])
                nc.vector.tensor_reduce(
                    parts[orow][:, t:t + 1], acc, AX.X, ALU.add)

        # ---------------- phase 2b: BCE + local u/v (from mc local) --------
        for ic in range(4):
            mct = mc_pool.tile([128, C], I32, tag="mcl")
            nc.sync.dma_start(out=mct, in_=mcl[ic * 128:(ic + 1) * 128, :])
            clt = bce_pool.tile([128, C], FP32, tag="clog")
            nc.sync.dma_start(out=clt, in_=clog[ic * 128:(ic + 1) * 128, :])


            mcft = bce_pool.tile([128, C], FP32, tag="mcft")
            nc.vector.tensor_copy(mcft, mct)
            mask = bce_pool.tile([128, C], BF16, tag="mask")
            tgt = bce_pool.tile([128, C], BF16, tag="tgt")
            nc.vector.tensor_scalar(
                mask, mcft, -1.0, None, ALU.not_equal, ALU.add,
                accum_out=parts[O_MS][:, ic:ic + 1])
            nc.vector.tensor_scalar(tgt, mcft, 0.0, None, ALU.max)
            sp = bce_pool.tile([128, C], FP32, tag="sp")
            nc.scalar.activation(sp, clt, ACTF.Exp)
            nc.scalar.activation(sp, sp, ACTF.Ln, bias=one_col)  # log1p(exp(x))
            scrB = bce_pool.tile([128, C], BF16, tag="scrB")
            nc.vector.scalar_tensor_tensor(
                scrB, mask, 1.0, sp, ALU.mult, ALU.mult,
                accum_out=parts[O_B1][:, ic:ic + 1])
            nc.vector.scalar_tensor_tensor(
                scrB, clt, 1.0, tgt, ALU.mult, ALU.mult,
                accum_out=parts[O_B2][:, ic:ic + 1])

        # ---------------- phase 4: Jaccard + KL main loop ----------------
        inv_t = float(1.0 / TEMP)
        for ic in range(4):
            zs_j = stats.tile([128, 4], FP32, tag="zs_j")
            es_j = stats.tile([128, 4], FP32, tag="es_j")
            ec_j = stats.tile([128, 4], FP32, tag="ec_j")
            zc_j = stats.tile([128, 4], FP32, tag="zc_j")
            cs_tiles = []
            for q4 in range(4):
                cst = cs_pool.tile([128, 1024], FP32, tag="cst")
                nc.sync.dma_start(
                    out=cst,
                    in_=csim[ic * 128:(ic + 1) * 128, q4 * 1024:(q4 + 1) * 1024])
                cs_tiles.append(cst)

            for jb in range(4):
                ups = ps_main.tile([128, 1024], FP32, tag="union")
                for g in range(2):
                    js0 = jb * 1024 + g * 512
                    opart = ups[:, g * 512:(g + 1) * 512]
                    rc0 = (jb * 1024 + g * 512) // 128
                    k = 0
                    for loc, full in ((nu4, u_T4), (nv4, v_T4)):
                        for cc in range(4):
                            nc.tensor.matmul(
                                opart,
                                loc[:, ic, cc, :],
                                full[:, rc0:rc0 + 4, cc, :],
                                start=(k == 0), stop=False)
                            k += 1
                    # + rs_j : ones(1x128) x rs_row(1x512)
                    nc.tensor.matmul(
                        opart, ones16[0:1, 0:128], rsrow_sb[0:1, js0:js0 + 512],
                        start=False, stop=False)
                    # + rs_i : rs_local(1x128) x ones(1x512)
                    nc.tensor.matmul(
                        opart, rsif16[0:1, ic * 128:(ic + 1) * 128],
                        ones16[0:1, 0:512],
                        start=False, stop=True)

                q = s3.tile([128, 1024], FP32, tag="q")
                nc.vector.reciprocal_approx_fast(out=q, in_=ups)
                sp1 = q  # in-place: sp1 = (rs_j + rs_i) * q overwrites q
                nc.vector.scalar_tensor_tensor(
                    sp1, rs_bcast[:, jb * 1024:(jb + 1) * 1024],
                    rsloc_sb[:, ic:ic + 1], q, ALU.add, ALU.mult)
                e = s3.tile([128, 1024], FP32, tag="e")
                nc.scalar.activation(
                    e, sp1, ACTF.Exp, bias=minvt_col, scale=inv_t,
                    accum_out=zs_j[:, jb:jb + 1])
                scr1 = scrp.tile([128, 1024], BF16, tag="scr1")
                nc.vector.scalar_tensor_tensor(
                    scr1, sp1, -1.0, e, ALU.add, ALU.mult,
                    accum_out=es_j[:, jb:jb + 1])
                scr2 = scrp.tile([128, 1024], BF16, tag="scr2")
                nc.vector.scalar_tensor_tensor(
                    scr2, cs_tiles[jb], 1.0, e, ALU.mult, ALU.mult,
                    accum_out=ec_j[:, jb:jb + 1])
                # csim exp-sum (reuse scr tile; only accumulator needed)
                scr3 = scrp.tile([128, 1024], BF16, tag="scr3")
                nc.scalar.activation(
                    scr3, cs_tiles[jb], ACTF.Exp,
                    accum_out=zc_j[:, jb:jb + 1])

            for src_t, orow in ((zs_j, O_ZS), (es_j, O_ES), (ec_j, O_EC),
                                (zc_j, O_ZC)):
                nc.vector.tensor_reduce(
                    parts[orow][:, ic:ic + 1], src_t, AX.X, ALU.add)

        # ---------------- outputs ----------------
        for k in range(NROWS):
            nc.sync.dma_start(
                out=out[k].rearrange("t p -> p t"), in_=parts[k])


_NC_CACHE = None
LAST_RESULT = None


def _get_nc():
    global _NC_CACHE
    if _NC_CACHE is None:
        _NC_CACHE = build_nc()
    return _NC_CACHE


def kernel(logits_per_image, logits_per_text, concepts_logits,
           concept_image_similarity, medical_concepts):
    img = np.ascontiguousarray(logits_per_image, dtype=np.float32)
    txt = np.ascontiguousarray(logits_per_text, dtype=np.float32)
    csim = np.ascontiguousarray(concept_image_similarity, dtype=np.float32)
    clog = np.ascontiguousarray(concepts_logits, dtype=np.float32)
    mc = np.ascontiguousarray(medical_concepts, dtype=np.int32)

    nc = _get_nc()
    in_maps = []
    for c in range(NCORES):
        g0 = c * R
        in_maps.append({
            "img": img[g0:g0 + R],
            "txt": txt[g0:g0 + R],
            "csim": csim[g0:g0 + R],
            "mcf": mc,
            "mcl": mc[g0:g0 + R],
            "clog": clog[g0:g0 + R],
        })
    res = run_bass_kernel_spmd(nc, in_maps, list(range(NCORES)))
    global LAST_RESULT
    LAST_RESULT = res
    outs = [r["out"].astype(np.float64).reshape(NROWS, 512) for r in res.results]

    # host finalization (all O(B))
    o = np.concatenate(outs, axis=1)  # [NROWS, B]
    zimg, ztxt, zc, zs, es, ec, b1, b2, ms = o

    diag_i = np.diagonal(img).astype(np.float64)
    diag_t = np.diagonal(txt).astype(np.float64)
    clip_loss = 0.5 * (np.mean(np.log(zimg) - diag_i)
                       + np.mean(np.log(ztxt) - diag_t))

    concept_loss = (b1.sum() - b2.sum()) / (ms.sum() + 1e-8)

    # kl_i = (ES_i/T)/Zs_i - log Zs_i - EC_i/Zs_i + log Zc_i
    kl = np.mean((es / TEMP) / zs - np.log(zs) - ec / zs + np.log(zc))

    total = clip_loss + CONCEPT_WEIGHT * concept_loss + CONCEPT_SIM_WEIGHT * kl
    return np.float32(total)


# revision 24
# speedup vs baseline: 1.0956x; 1.0956x over previous
"""Trainium2 Bass kernel for nn_CCALoss (CLIP + masked concept BCE + Jaccard-KL loss).

Contract: kernel(**inputs) takes the FULL unsharded inputs and returns the FULL
(scalar) output. Internally shards the batch dim across 8 NeuronCores; each core
computes per-row partial sums; the host does the O(B) finalization in fp64.

Per-core device work (R = 512 local rows, B = 4096, C = 512):
  - Zimg/Ztxt:  row-wise sum(exp(x)) of logits_per_image/text local rows
                (ScalarE exp with fused accum_out; lse computed on host).
  - BCE:        masked softplus sums over concepts for local rows
                (exp + log1p on ScalarE, fused STT dot-accumulate on VectorE).
  - Jaccard:    inter/union via two GEMMs over u=(mc!=0), v=(mc==1) in bf16 on
                TensorE. union = rs_i + rs_j - inter accumulated directly in
                PSUM via K=1 fp16 fold matmuls. q = 1/union (custom DVE recip),
                s' = (rs_i+rs_j)*q - 1, e = exp(s'/T) (ScalarE, accum -> Zs),
                ES = sum e*s' and EC = sum e*csim via fused STT accumulates.
"""

import numpy as np

import concourse.bacc as bacc
import concourse.bass as bass
import concourse.tile as tile
from concourse import mybir
from concourse.bass_utils import run_bass_kernel_spmd

B = 4096
C = 512
NCORES = 8
R = B // NCORES  # 512 rows per core
TEMP = 0.07
CONCEPT_WEIGHT = 0.5
CONCEPT_SIM_WEIGHT = 0.3

FP32 = mybir.dt.float32
FP8 = mybir.dt.float8e4
FP16 = mybir.dt.float16
BF16 = mybir.dt.bfloat16
I32 = mybir.dt.int32
AX = mybir.AxisListType
ALU = mybir.AluOpType
ACTF = mybir.ActivationFunctionType

# output rows in the [NROWS, 4, 128] per-core output tensor
O_ZIMG, O_ZTXT, O_ZC, O_ZS, O_ES, O_EC, O_B1, O_B2, O_MS = range(9)
NROWS = 9


def build_nc():
    nc = bacc.Bacc("TRN2", target_bir_lowering=False, debug=False)

    img = nc.dram_tensor("img", [R, B], FP32, kind="ExternalInput")
    txt = nc.dram_tensor("txt", [R, B], FP32, kind="ExternalInput")
    csim = nc.dram_tensor("csim", [R, B], FP32, kind="ExternalInput")
    mcf = nc.dram_tensor("mcf", [B, C], I32, kind="ExternalInput")
    mcl = nc.dram_tensor("mcl", [R, C], I32, kind="ExternalInput")
    clog = nc.dram_tensor("clog", [R, C], FP32, kind="ExternalInput")
    out = nc.dram_tensor("out", [NROWS, 4, 128], FP32, kind="ExternalOutput")

    # internal DRAM scratch for partition-major <-> free-major relayouts
    rs_d = nc.dram_tensor("rs_scratch", [B], FP16)
    rsl_d = nc.dram_tensor("rsl_scratch", [R], FP32)

    with tile.TileContext(nc) as tc:
        _build(nc, tc, img, txt, csim, mcf, mcl, clog, out, rs_d, rsl_d)
    nc.compile()
    return nc


def _build(nc, tc, img, txt, csim, mcf, mcl, clog, out, rs_d, rsl_d):
    from contextlib import ExitStack

    ctx = ExitStack()
    with ctx:
        singles = ctx.enter_context(tc.tile_pool(name="singles", bufs=1))
        mc_pool = ctx.enter_context(tc.tile_pool(name="mc", bufs=6))
        big = ctx.enter_context(tc.tile_pool(name="big", bufs=3))
        cs_pool = ctx.enter_context(tc.tile_pool(name="cs", bufs=5))
        s3 = ctx.enter_context(tc.tile_pool(name="s3", bufs=3))
        scrp = ctx.enter_context(tc.tile_pool(name="scr", bufs=1))
        bce_pool = ctx.enter_context(tc.tile_pool(name="bce", bufs=1))
        stats = ctx.enter_context(tc.tile_pool(name="stats", bufs=1))

        # ---------------- constants ----------------
        ones16 = singles.tile([128, 512], FP16)
        nc.vector.memset(ones16, 1.0)
        mones_bf = singles.tile([128, 1], BF16)
        nc.vector.memset(mones_bf, -1.0)
        halves_bf = singles.tile([128, 1], BF16)
        nc.vector.memset(halves_bf, 0.5)
        one_col = singles.tile([128, 1], FP32)
        nc.vector.memset(one_col, 1.0)
        minvt_col = singles.tile([128, 1], FP32)
        nc.vector.memset(minvt_col, -float(1.0 / TEMP))

        # persistent big tensors
        # u_T4[p, rc, cc, j] = u[rc*128+j, cc*128+p]
        u_T4 = singles.tile([128, 32, 4, 128], BF16)
        v_T4 = singles.tile([128, 32, 4, 128], BF16)
        nu4 = singles.tile([128, 4, 4, 128], BF16)  # -0.5 * u_local^T
        nv4 = singles.tile([128, 4, 4, 128], BF16)
        rsrow_sb = singles.tile([1, B], FP16)  # free-major rs (all j)
        rsloc_sb = singles.tile([128, 4], FP32)  # rs of local rows, part-major
        rsif = singles.tile([1, R], FP32)  # rs of local rows, free-major
        rsif16 = singles.tile([1, R], FP16)

        # per-row stats tiles (partition-major, col = row-tile index)
        parts = {
            k: stats.tile([128, 4], FP32, tag=f"p{k}", name=f"parts{k}")
            for k in range(NROWS)
        }

        # ---------------- phase 1: u_T / v_T from full mc ----------------
        # u = (mc != 0), v = (mc == 1) as bf16 {0,1}; per-row-chunk sbuf
        # xbar transposes into contiguous [4,128] dests. Bacc's
        # generate_event_semaphores pass collapses multi-waits so the
        # single-wait-slot XPOSE instructions lower fine.
        uvpre = ctx.enter_context(tc.tile_pool(name="uvpre", bufs=4))
        for ic in range(4):
            mct = mc_pool.tile([128, C], I32, tag="mcf", name=f"mcln{ic}")
            nc.sync.dma_start(out=mct, in_=mcl[ic * 128:(ic + 1) * 128, :])
            nup = uvpre.tile([128, C], BF16, tag="up", name=f"nup{ic}")
            nvp = uvpre.tile([128, C], BF16, tag="vp", name=f"nvp{ic}")
            nc.vector.tensor_scalar(nup, mct, 0, -0.5, ALU.not_equal, ALU.mult)
            nc.vector.tensor_scalar(nvp, mct, 1, -0.5, ALU.is_equal, ALU.mult)
            nc.sync.dma_start_transpose(out=nu4[:, ic], in_=nup)
            nc.scalar.dma_start_transpose(out=nv4[:, ic], in_=nvp)
        for rc in range(32):
            mct = mc_pool.tile([128, C], I32, tag="mcf", name=f"mcf{rc}")
            nc.sync.dma_start(out=mct, in_=mcf[rc * 128:(rc + 1) * 128, :])
            up = uvpre.tile([128, C], BF16, tag="up", name=f"up{rc}")
            vp = uvpre.tile([128, C], BF16, tag="vp", name=f"vp{rc}")
            nc.vector.tensor_scalar(up, mct, 0, None, ALU.not_equal)
            nc.vector.tensor_scalar(vp, mct, 0, None, ALU.max)
            nc.sync.dma_start_transpose(out=u_T4[:, rc], in_=up)
            nc.scalar.dma_start_transpose(out=v_T4[:, rc], in_=vp)

        # ---------------- phase 3: row-sum vectors rs ----------------
        with tc.tile_pool(name="psB", bufs=2, space="PSUM") as ps_rs:
            # rs_loc[i] (partition-major) = sum_c 0.5*(u+v) for local rows
            for ic in range(4):
                ps = ps_rs.tile([128, 1], FP32, tag="rsloc", name=f"rslc{ic}")
                k = 0
                for loc in (nu4, nv4):
                    for cc in range(4):
                        nc.tensor.matmul(
                            ps, loc[:, ic, cc, :], mones_bf,
                            start=(k == 0), stop=(k == 7))
                        k += 1
                nc.scalar.copy(rsloc_sb[:, ic:ic + 1], ps)
            # relayout partition-major -> free-major through DRAM
            nc.gpsimd.dma_start(
                out=rsl_d.ap().rearrange("(t p) -> p t", p=128), in_=rsloc_sb)
            nc.gpsimd.dma_start(
                out=rsif, in_=rsl_d.ap().rearrange("(o x) -> o x", o=1))
            nc.vector.tensor_copy(rsif16, rsif)

            # rs_row[j] for all 4096 j (free-major): ones-reduce over u_T/v_T
            for js in range(8):
                ps = ps_rs.tile([1, 512], FP32, tag="rsrow", name=f"rsrw{js}")
                k = 0
                for tens in (u_T4, v_T4):
                    for cc in range(4):
                        nc.tensor.matmul(
                            ps, halves_bf, tens[:, 4 * js:4 * js + 4, cc, :],
                            start=(k == 0), stop=(k == 7))
                        k += 1
                nc.scalar.copy(rsrow_sb[:, js * 512:(js + 1) * 512], ps)

        # ---------------- phase 2: img/txt exp-sum streams ----------------
        for src, orow in ((img, O_ZIMG), (txt, O_ZTXT)):
            for t in range(4):
                acc = stats.tile([128, 2], FP32, tag="zacc")
                for h in range(2):
                    tl = big.tile([128, 2048], FP32, tag="imgtxt")
                    nc.sync.dma_start(
                        out=tl,
                        in_=src[t * 128:(t + 1) * 128, h * 2048:(h + 1) * 2048],
                    )
                    # in-place exp; we only need the row-sum accumulator
                    nc.scalar.activation(tl, tl, ACTF.Exp, accum_out=acc[:, h:h + 1])
                nc.vector.tensor_reduce(
                    parts[orow][:, t:t + 1], acc, AX.X, ALU.add)

        # ---------------- phase 2b: BCE + local u/v (from mc local) --------
        for ic in range(4):
            mct = mc_pool.tile([128, C], I32, tag="mcl")
            nc.sync.dma_start(out=mct, in_=mcl[ic * 128:(ic + 1) * 128, :])
            clt = bce_pool.tile([128, C], FP32, tag="clog")
            nc.sync.dma_start(out=clt, in_=clog[ic * 128:(ic + 1) * 128, :])


            mcft = bce_pool.tile([128, C], FP32, tag="mcft")
            nc.vector.tensor_copy(mcft, mct)
            mask = bce_pool.tile([128, C], BF16, tag="mask")
            tgt = bce_pool.tile([128, C], BF16, tag="tgt")
            nc.vector.tensor_scalar(
                mask, mcft, -1.0, None, ALU.not_equal, ALU.add,
                accum_out=parts[O_MS][:, ic:ic + 1])
            nc.vector.tensor_scalar(tgt, mcft, 0.0, None, ALU.max)
            sp = bce_pool.tile([128, C], FP32, tag="sp")
            nc.scalar.activation(sp, clt, ACTF.Exp)
            nc.scalar.activation(sp, sp, ACTF.Ln, bias=one_col)  # log1p(exp(x))
            scrB = bce_pool.tile([128, C], BF16, tag="scrB")
            nc.vector.scalar_tensor_tensor(
                scrB, mask, 1.0, sp, ALU.mult, ALU.mult,
                accum_out=parts[O_B1][:, ic:ic + 1])
            nc.vector.scalar_tensor_tensor(
                scrB, clt, 1.0, tgt, ALU.mult, ALU.mult,
                accum_out=parts[O_B2][:, ic:ic + 1])

        # ---------------- phase 4: Jaccard + KL main loop ----------------
        ps_main = ctx.enter_context(tc.tile_pool(name="psA", bufs=2, space="PSUM"))
        ps_ab = ctx.enter_context(tc.tile_pool(name="psC", bufs=2, space="PSUM"))
        inv_t = float(1.0 / TEMP)
        for ic in range(4):
            zs_j = stats.tile([128, 4], FP32, tag="zs_j")
            es_j = stats.tile([128, 4], FP32, tag="es_j")
            ec_j = stats.tile([128, 4], FP32, tag="ec_j")
            zc_j = stats.tile([128, 4], FP32, tag="zc_j")
            cs_tiles = []
            for q4 in range(4):
                cst = cs_pool.tile([128, 1024], FP32, tag="cst")
                nc.sync.dma_start(
                    out=cst,
                    in_=csim[ic * 128:(ic + 1) * 128, q4 * 1024:(q4 + 1) * 1024])
                cs_tiles.append(cst)

            for jb in range(4):
                ups = ps_main.tile([128, 1024], FP32, tag="union")
                abps = ps_ab.tile([128, 1024], FP32, tag="ab")
                for g in range(2):
                    js0 = jb * 1024 + g * 512
                    opart = ups[:, g * 512:(g + 1) * 512]
                    abpart = abps[:, g * 512:(g + 1) * 512]
                    # ab = rs_i + rs_j (rank-1, via two K=1 fp16 matmuls)
                    nc.tensor.matmul(
                        abpart, ones16[0:1, 0:128], rsrow_sb[0:1, js0:js0 + 512],
                        start=True, stop=False)
                    nc.tensor.matmul(
                        abpart, rsif16[0:1, ic * 128:(ic + 1) * 128],
                        ones16[0:1, 0:512],
                        start=False, stop=True)
                    rc0 = (jb * 1024 + g * 512) // 128
                    k = 0
                    for loc, full in ((nu4, u_T4), (nv4, v_T4)):
                        for cc in range(4):
                            nc.tensor.matmul(
                                opart,
                                loc[:, ic, cc, :],
                                full[:, rc0:rc0 + 4, cc, :],
                                start=(k == 0), stop=False)
                            k += 1
                    # + rs_j : ones(1x128) x rs_row(1x512)
                    nc.tensor.matmul(
                        opart, ones16[0:1, 0:128], rsrow_sb[0:1, js0:js0 + 512],
                        start=False, stop=False)
                    # + rs_i : rs_local(1x128) x ones(1x512)
                    nc.tensor.matmul(
                        opart, rsif16[0:1, ic * 128:(ic + 1) * 128],
                        ones16[0:1, 0:512],
                        start=False, stop=True)

                q = s3.tile([128, 1024], FP32, tag="q")
                nc.vector.reciprocal_approx_fast(out=q, in_=ups)
                sp1 = q  # in-place: sp1 = (rs_i + rs_j) * q overwrites q
                nc.vector.scalar_tensor_tensor(
                    sp1, abps, 1.0, q, ALU.mult, ALU.mult)
                e = s3.tile([128, 1024], FP32, tag="e")
                nc.scalar.activation(
                    e, sp1, ACTF.Exp, bias=minvt_col, scale=inv_t,
                    accum_out=zs_j[:, jb:jb + 1])
                scr1 = scrp.tile([128, 1024], BF16, tag="scr1")
                nc.vector.scalar_tensor_tensor(
                    scr1, sp1, -1.0, e, ALU.add, ALU.mult,
                    accum_out=es_j[:, jb:jb + 1])
                scr2 = scrp.tile([128, 1024], BF16, tag="scr2")
                nc.vector.scalar_tensor_tensor(
                    scr2, cs_tiles[jb], 1.0, e, ALU.mult, ALU.mult,
                    accum_out=ec_j[:, jb:jb + 1])
                # csim exp-sum (reuse scr tile; only accumulator needed)
                scr3 = scrp.tile([128, 1024], BF16, tag="scr3")
                nc.scalar.activation(
                    scr3, cs_tiles[jb], ACTF.Exp,
                    accum_out=zc_j[:, jb:jb + 1])

            for src_t, orow in ((zs_j, O_ZS), (es_j, O_ES), (ec_j, O_EC),
                                (zc_j, O_ZC)):
                nc.vector.tensor_reduce(
                    parts[orow][:, ic:ic + 1], src_t, AX.X, ALU.add)

        # ---------------- outputs ----------------
        for k in range(NROWS):
            nc.sync.dma_start(
                out=out[k].rearrange("t p -> p t"), in_=parts[k])


_NC_CACHE = None
LAST_RESULT = None


def _get_nc():
    global _NC_CACHE
    if _NC_CACHE is None:
        _NC_CACHE = build_nc()
    return _NC_CACHE


def kernel(logits_per_image, logits_per_text, concepts_logits,
           concept_image_similarity, medical_concepts):
    img = np.ascontiguousarray(logits_per_image, dtype=np.float32)
    txt = np.ascontiguousarray(logits_per_text, dtype=np.float32)
    csim = np.ascontiguousarray(concept_image_similarity, dtype=np.float32)
    clog = np.ascontiguousarray(concepts_logits, dtype=np.float32)
    mc = np.ascontiguousarray(medical_concepts, dtype=np.int32)

    nc = _get_nc()
    in_maps = []
    for c in range(NCORES):
        g0 = c * R
        in_maps.append({
            "img": img[g0:g0 + R],
            "txt": txt[g0:g0 + R],
            "csim": csim[g0:g0 + R],
            "mcf": mc,
            "mcl": mc[g0:g0 + R],
            "clog": clog[g0:g0 + R],
        })
    res = run_bass_kernel_spmd(nc, in_maps, list(range(NCORES)))
    global LAST_RESULT
    LAST_RESULT = res
    outs = [r["out"].astype(np.float64).reshape(NROWS, 512) for r in res.results]

    # host finalization (all O(B))
    o = np.concatenate(outs, axis=1)  # [NROWS, B]
    zimg, ztxt, zc, zs, es, ec, b1, b2, ms = o

    diag_i = np.diagonal(img).astype(np.float64)
    diag_t = np.diagonal(txt).astype(np.float64)
    clip_loss = 0.5 * (np.mean(np.log(zimg) - diag_i)
                       + np.mean(np.log(ztxt) - diag_t))

    concept_loss = (b1.sum() - b2.sum()) / (ms.sum() + 1e-8)

    # kl_i = (ES_i/T)/Zs_i - log Zs_i - EC_i/Zs_i + log Zc_i
    kl = np.mean((es / TEMP) / zs - np.log(zs) - ec / zs + np.log(zc))

    total = clip_loss + CONCEPT_WEIGHT * concept_loss + CONCEPT_SIM_WEIGHT * kl
    return np.float32(total)
